# revision 15
# baseline (speedup 1.0000x reference)
"""ALiBi multi-head attention (B=4, Tq=1024, D=1024, H=16, cache=1024) on 8
Trainium2 NeuronCores.

Sharding: core c = (batch b = c//2, head-group g = c%2). Each core runs one
batch with 8 of the 16 heads. Heads are assigned to groups interleaved by
ALiBi window size so per-core work balances, and all cores run an identical
SPMD graph (per-slot key windows are the max over the two groups).

Device dataflow (all transposed so ALiBi becomes a per-partition ACT bias):
  qT = Wq_s^T x^T + bq        kTnew = Wk_s^T x^T        vnew = x Wv_s + bv
  scoresT[k,q] = kT_chunk^T qT        (K=64 contraction, 2-head row packing)
  P^T = exp(0.125*scoresT + alibi[k])  (single ScalarE pass, PSUM->SBUF bf16)
  oT += v_chunk^T P^T  (V stationary, col-packed pairs -> PSUM[128,1024])
  denom += ones^T P^T  (M=64 ones lhsT -> denominator broadcast to 64 rows)
  wvT = oT * recip(denom);  outT_partial = Wo_s^T wvT + 0.5*bo
Softmax max-subtraction is skipped (scores are O(1), bias <= 0) and keys with
alibi bias < -16 are dropped (error ~1e-7 << tolerance).

Host: shards/pre-transposes inputs (bf16), sums the two partial outT per
batch, scatters head-sliced k/v outputs. No collectives on device.
"""

import numpy as np
import ml_dtypes

BF16 = ml_dtypes.bfloat16

B, Tq, D = 4, 1024, 1024
H, DH = 16, 64
CACHE = 1024
Tk = CACHE + Tq
NCH = Tk // 128  # 16 key chunks
T_CUT = 16.0

# ---- head assignment / windows (hardcoded, deterministic) ----
_slopes = 2.0 ** (-(8.0 / H) * np.arange(1, H + 1))  # head h=0..15 -> slope
_raw = np.minimum(np.ceil(T_CUT / _slopes), Tk)
_W = np.minimum(((_raw + 127) // 128) * 128, Tk).astype(int)
_order = np.argsort(-_W, kind="stable")
SLOT_W = [int(max(_W[_order[2 * s]], _W[_order[2 * s + 1]])) for s in range(8)]
HEADS_OF_GROUP = [[int(_order[2 * s + g]) for s in range(8)] for g in (0, 1)]
COLS_G = [
    np.concatenate([np.arange(h * DH, (h + 1) * DH) for h in HEADS_OF_GROUP[g]])
    for g in (0, 1)
]

_COMPILED = None


def _build():
    import concourse.bacc as bacc
    import concourse.tile as tile
    import concourse.mybir as mybir

    f32 = mybir.dt.float32
    bf16 = mybir.dt.bfloat16
    ADD = mybir.AluOpType.add
    MUL = mybir.AluOpType.mult
    EXP = mybir.ActivationFunctionType.Exp

    nc = bacc.Bacc("TRN2", target_bir_lowering=False, debug=False,
                   num_swdge_queues=4)

    # all inputs pre-arranged on host to [128, free] partition-major layouts
    xt = nc.dram_tensor("xt", [128, 8 * Tq], bf16, kind="ExternalInput")
    wq = nc.dram_tensor("wq", [128, 8 * 512], bf16, kind="ExternalInput")
    wk = nc.dram_tensor("wk", [128, 8 * 512], bf16, kind="ExternalInput")
    wv = nc.dram_tensor("wv", [128, 8 * 512], bf16, kind="ExternalInput")
    wo = nc.dram_tensor("wo", [128, 4 * Tq], bf16, kind="ExternalInput")
    pbf = nc.dram_tensor("pbf", [128, 2048], bf16, kind="ExternalInput")  # vc0|vc1|kct
    pf32 = nc.dram_tensor("pf32", [128, 652], f32, kind="ExternalInput")  # bq|bo|alibi|bvb

    ko = nc.dram_tensor("ko", [512, Tq], bf16, kind="ExternalOutput")
    vo = nc.dram_tensor("vo", [Tq, 512], bf16, kind="ExternalOutput")
    oo = nc.dram_tensor("oo", [D, Tq], bf16, kind="ExternalOutput")

    ko_r = ko[:].rearrange("(c p) t -> p c t", p=128)
    vo_r = vo[:].rearrange("(c p) (s e) -> p c s e", p=128, s=8)
    oo_r = oo[:].rearrange("(c p) t -> p c t", p=128)

    with tile.TileContext(nc) as tc:
        with (
            tc.tile_pool(name="const", bufs=1) as cp,
            tc.tile_pool(name="pt", bufs=4) as ptp,
            tc.tile_pool(name="rc", bufs=2) as rcp,
            tc.tile_pool(name="bc", bufs=2) as bcp,
            tc.tile_pool(name="sc", bufs=2, space="PSUM") as scp,
            tc.tile_pool(name="proj", bufs=1, space="PSUM") as projp,
            tc.tile_pool(name="otp", bufs=1, space="PSUM") as otp,
        ):
            # resident SBUF tensors
            xt_sb = cp.tile([128, 8, Tq], bf16)
            wq_sb = cp.tile([128, 8, 512], bf16)
            wk_sb = cp.tile([128, 8, 512], bf16)
            wv_sb = cp.tile([128, 8, 512], bf16)
            wo_sb = cp.tile([128, 4, Tq], bf16)
            pbf_sb = cp.tile([128, 2048], bf16)
            pf32_sb = cp.tile([128, 652], f32)
            vc_sb = [cp.tile([128, 8, DH + 1], bf16, name=f"vc_sb{i}") for i in (0, 1)]
            qt_sb = cp.tile([128, 4, Tq], bf16)
            kt_sb = cp.tile([128, 4, Tq], bf16)
            v_sb = cp.tile([128, 8, 8, DH + 1], bf16)
            wvt_sb = cp.tile([128, 4, Tq], bf16)
            ot_sb = cp.tile([128, 8, Tq], bf16)

            # zero-copy views into the packed tiles
            kct_sb = pbf_sb[:, 1024:2048]
            bq_sb = pf32_sb[:, 0:4]
            bo_sb = pf32_sb[:, 4:12]
            alibi_sb = pf32_sb[:, 12:140].rearrange("p (s j) -> p s j", s=8)
            bvb_sb = pf32_sb[:, 140:652]

            # loads: critical path first, spread across engine queues
            nc.sync.dma_start(out=xt_sb[:, 0:4, :], in_=xt[:, 0:4 * Tq].rearrange("p (c t) -> p c t", c=4))
            nc.scalar.dma_start(out=xt_sb[:, 4:8, :], in_=xt[:, 4 * Tq:].rearrange("p (c t) -> p c t", c=4))
            nc.gpsimd.dma_start(out=wq_sb[:], in_=wq[:].rearrange("p (c n) -> p c n", c=8))
            nc.gpsimd.dma_start(out=pf32_sb[:], in_=pf32[:])
            nc.gpsimd.dma_start(out=pbf_sb[:], in_=pbf[:])
            nc.scalar.dma_start(out=wk_sb[:], in_=wk[:].rearrange("p (c n) -> p c n", c=8))
            nc.gpsimd.dma_start(out=wv_sb[:], in_=wv[:].rearrange("p (c n) -> p c n", c=8))
            nc.sync.dma_start(out=wo_sb[:], in_=wo[:].rearrange("p (c n) -> p c n", c=4))
            # unpack cached V into the ones-augmented layout
            for i in (0, 1):
                nc.vector.tensor_copy(
                    out=vc_sb[i][:, :, 0:DH],
                    in_=pbf_sb[:, i * 512:(i + 1) * 512].rearrange("p (s e) -> p s e", s=8))
            # ones columns for the augmented V (softmax denominators)
            nc.vector.memset(v_sb[:, :, :, DH:DH + 1], 1.0)
            nc.vector.memset(vc_sb[0][:, :, DH:DH + 1], 1.0)
            nc.vector.memset(vc_sb[1][:, :, DH:DH + 1], 1.0)

            # ---- qT projection (uses "sc" psum tag; attention not started) ----
            for m in range(4):
                ps = scp.tile([128, 1024], f32, tag="sc")
                for qh in range(2):
                    for kc in range(8):
                        nc.tensor.matmul(
                            ps[:, qh * 512:(qh + 1) * 512],
                            lhsT=wq_sb[:, kc, m * 128:(m + 1) * 128],
                            rhs=xt_sb[:, kc, qh * 512:(qh + 1) * 512],
                            start=(kc == 0), stop=(kc == 7),
                        )
                nc.vector.tensor_scalar(qt_sb[:, m, :], ps[:], bq_sb[:, m:m + 1], None, ADD)

            # ---- filler generators: kT / v projections, pumped during attention ----
            def kt_gen(m):
                ps = projp.tile([128, 1024], f32, tag="proj")
                for qh in range(2):
                    for kc in range(8):
                        nc.tensor.matmul(
                            ps[:, qh * 512:(qh + 1) * 512],
                            lhsT=wk_sb[:, kc, m * 128:(m + 1) * 128],
                            rhs=xt_sb[:, kc, qh * 512:(qh + 1) * 512],
                            start=(kc == 0), stop=(kc == 7),
                        )
                        yield
                nc.vector.tensor_copy(out=kt_sb[:, m, :], in_=ps[:])
                nc.sync.dma_start(out=ko_r[:, m, :], in_=kt_sb[:, m, :])
                yield

            def v_gen(t8):
                ps = projp.tile([128, 1024], f32, tag="proj")
                for kc in range(8):
                    nc.tensor.matmul(
                        ps[:, :512],
                        lhsT=xt_sb[:, kc, t8 * 128:(t8 + 1) * 128],
                        rhs=wv_sb[:, kc, :],
                        start=(kc == 0), stop=(kc == 7),
                    )
                    yield
                nc.vector.tensor_tensor(
                    v_sb[:, t8, :, 0:DH],
                    ps[:, :512].rearrange("p (s e) -> p s e", s=8),
                    bvb_sb[:].rearrange("p (s e) -> p s e", s=8), ADD)
                nc.sync.dma_start(out=vo_r[:, t8, :, :], in_=v_sb[:, t8, :, 0:DH])
                yield

            filler = [kt_gen(0)] + [v_gen(t) for t in range(8)] + \
                     [kt_gen(1), kt_gen(2), kt_gen(3)]

            def pump(n):
                while n > 0 and filler:
                    try:
                        next(filler[0])
                        n -= 1
                    except StopIteration:
                        filler.pop(0)

            # ---- attention: slot-major, AV lags exp by 2 items ----
            for p in range(4):
                for d in (0, 1):
                    s = 2 * p + d
                    W = SLOT_W[s]
                    j_first = NCH - W // 128
                    if s <= 1 or s % 2 == 1:
                        ot = otp.tile([65, 1024], f32, tag="ot")
                    else:
                        ot = projp.tile([65, 1024], f32, tag="proj")

                    def av_emit(j, pt, ot=ot, s=s, p=p, d=d, j_first=j_first):
                        if j < 8:
                            vsrc = vc_sb[s][:, j, :]
                        else:
                            vsrc = v_sb[:, j - 8, s, :]
                        for qh in range(2):
                            nc.tensor.matmul(
                                ot[:, qh * 512:(qh + 1) * 512],
                                lhsT=vsrc,
                                rhs=pt[:, qh * 512:(qh + 1) * 512],
                                start=(j == j_first), stop=(j == NCH - 1),
                                skip_group_check=True,
                            )

                    pend = []
                    for j in range(j_first, NCH):
                        sc = scp.tile([128, 1024], f32, tag="sc")
                        if j < 8:
                            klh = kct_sb[d * 64:(d + 1) * 64, j * 128:(j + 1) * 128]
                        else:
                            klh = kt_sb[d * 64:(d + 1) * 64, p, (j - 8) * 128:(j - 7) * 128]
                        for qh in range(2):
                            nc.tensor.matmul(
                                sc[:, qh * 512:(qh + 1) * 512],
                                lhsT=klh,
                                rhs=qt_sb[d * 64:(d + 1) * 64, p, qh * 512:(qh + 1) * 512],
                                start=True, stop=True,
                            )
                        pt = ptp.tile([128, 1024], bf16, tag="pt")
                        nc.scalar.activation(pt[:], sc[:], EXP,
                                             bias=alibi_sb[:, s, j:j + 1], scale=0.125)
                        pump(6)
                        pend.append((j, pt))
                        if len(pend) > 2:
                            av_emit(*pend.pop(0))
                    for it in pend:
                        av_emit(*it)

                    # normalize: wvT rows = oT * recip(denom row).
                    # DVE rules: tensor_copy may shift base partitions; the
                    # custom recip and 2-input tensor_tensor must be aligned.
                    rc = rcp.tile([1, 1024], f32, tag="rc")
                    rcr = rcp.tile([1, 1024], f32, tag="rcr")
                    nc.vector.tensor_copy(out=rc[:], in_=ot[64:65, :])
                    nc.vector.reciprocal_approx_fast(rcr[:], rc[:])
                    bc = bcp.tile([64, 1024], f32, tag="bc")
                    nc.gpsimd.partition_broadcast(bc[:], rcr[:])
                    if d == 0:
                        nc.vector.tensor_tensor(
                            wvt_sb[0:64, p, :], ot[0:64, :], bc[:], MUL)
                    else:
                        tmp = bcp.tile([64, 1024], bf16, tag="ntmp")
                        nc.vector.tensor_tensor(tmp[:], ot[0:64, :], bc[:], MUL)
                        nc.vector.tensor_copy(out=wvt_sb[64:128, p, :], in_=tmp[:])

            # ---- out projection ----
            for m in range(8):
                ps = scp.tile([128, 1024], f32, tag="sc")
                for qh in range(2):
                    for kc in range(4):
                        nc.tensor.matmul(
                            ps[:, qh * 512:(qh + 1) * 512],
                            lhsT=wo_sb[:, kc, m * 128:(m + 1) * 128],
                            rhs=wvt_sb[:, kc, qh * 512:(qh + 1) * 512],
                            start=(kc == 0), stop=(kc == 3),
                        )
                nc.vector.tensor_scalar(ot_sb[:, m, :], ps[:], bo_sb[:, m:m + 1], None, ADD)
                nc.sync.dma_start(out=oo_r[:, m, :], in_=ot_sb[:, m, :])

    nc.compile()
    return nc


def _get_compiled():
    global _COMPILED
    if _COMPILED is None:
        _COMPILED = _build()
    return _COMPILED


def _reference_numpy(x, k_cache, v_cache, mask, Wq, bq, Wk, Wv, bv, Wo, bo):
    """Exact numpy fallback (used only if mask is nonzero)."""
    q = x @ Wq + bq
    k = np.concatenate([k_cache, x @ Wk], axis=1)
    v = np.concatenate([v_cache, x @ Wv + bv], axis=1)
    kn, vn = k[:, -CACHE:, :], v[:, -CACHE:, :]
    qh = q.reshape(B, Tq, H, DH).transpose(0, 2, 1, 3)
    kh = k.reshape(B, Tk, H, DH).transpose(0, 2, 1, 3)
    vh = v.reshape(B, Tk, H, DH).transpose(0, 2, 1, 3)
    slopes = 2.0 ** (-(8.0 / H) * np.arange(1, H + 1))
    rel = np.arange(Tk - 1, -1, -1, dtype=np.float32)
    bias = (-(slopes[:, None] * rel[None, :])).astype(np.float32)[None, :, None, :]
    scores = np.einsum("bhqd,bhkd->bhqk", qh, kh) / np.sqrt(DH) + mask + bias
    scores -= scores.max(axis=-1, keepdims=True)
    e = np.exp(scores)
    attn = e / e.sum(axis=-1, keepdims=True)
    a = np.einsum("bhqk,bhkd->bhqd", attn, vh)
    out = a.transpose(0, 2, 1, 3).reshape(B, Tq, D) @ Wo + bo
    return (out.astype(np.float32), kn.astype(np.float32), vn.astype(np.float32))


def _make_in_maps(inputs):
    x = np.asarray(inputs["x"], np.float32)
    k_cache = np.asarray(inputs["k_cache"], np.float32)
    v_cache = np.asarray(inputs["v_cache"], np.float32)
    Wq, bq = np.asarray(inputs["Wq"], np.float32), np.asarray(inputs["bq"], np.float32)
    Wk = np.asarray(inputs["Wk"], np.float32)
    Wv, bv = np.asarray(inputs["Wv"], np.float32), np.asarray(inputs["bv"], np.float32)
    Wo, bo = np.asarray(inputs["Wo"], np.float32), np.asarray(inputs["bo"], np.float32)

    def pmajor(a, nch):
        # (nch*128, F) -> (128, nch*F) partition-major
        F = a.shape[1]
        return np.ascontiguousarray(
            a.reshape(nch, 128, F).transpose(1, 0, 2).reshape(128, nch * F))

    alibi_g, pf32_g = [], []
    for g in (0, 1):
        heads = HEADS_OF_GROUP[g]
        al = np.empty((128, 8, NCH), np.float32)
        kpos = np.arange(128)
        for s in range(8):
            sl = _slopes[heads[s]]
            for j in range(NCH):
                al[:, s, j] = -sl * (Tk - 1 - (j * 128 + kpos))
        alibi_g.append(al)
        cols = COLS_G[g]
        pf = np.empty((128, 652), np.float32)
        pf[:, 0:4] = bq[cols].reshape(4, 128).T
        pf[:, 4:12] = (0.5 * bo).reshape(8, 128).T
        pf[:, 12:140] = al.reshape(128, 128)
        pf[:, 140:652] = np.broadcast_to(bv[cols], (128, 512))
        pf32_g.append(pf)

    in_maps = []
    for c in range(8):
        b, g = c // 2, c % 2
        heads = HEADS_OF_GROUP[g]
        cols = COLS_G[g]
        kct_arr = np.concatenate(
            [k_cache[b][:, heads[s] * DH:(heads[s] + 1) * DH].T for s in (0, 1)], axis=0
        ).astype(BF16)  # (128, 1024)
        pb = np.empty((128, 2048), BF16)
        for i in (0, 1):
            vcs = v_cache[b][:, heads[i] * DH:(heads[i] + 1) * DH].astype(BF16)  # (1024, 64)
            pb[:, i * 512:(i + 1) * 512] = (
                vcs.reshape(8, 128, DH).transpose(1, 0, 2).reshape(128, 512))
        pb[:, 1024:2048] = kct_arr
        in_maps.append({
            "xt": pmajor(np.ascontiguousarray(x[b].T).astype(BF16), 8),
            "wq": pmajor(Wq[:, cols].astype(BF16), 8),
            "wk": pmajor(Wk[:, cols].astype(BF16), 8),
            "wv": pmajor(Wv[:, cols].astype(BF16), 8),
            "wo": pmajor(Wo[cols, :].astype(BF16), 4),
            "pbf": pb,
            "pf32": pf32_g[g],
        })
    return in_maps


def kernel(x, k_cache, v_cache, mask, Wq, bq, Wk, Wv, bv, Wo, bo):
    mask = np.asarray(mask, np.float32)
    if np.any(mask):
        return _reference_numpy(
            np.asarray(x, np.float32), np.asarray(k_cache, np.float32),
            np.asarray(v_cache, np.float32), mask,
            np.asarray(Wq, np.float32), np.asarray(bq, np.float32),
            np.asarray(Wk, np.float32), np.asarray(Wv, np.float32),
            np.asarray(bv, np.float32), np.asarray(Wo, np.float32),
            np.asarray(bo, np.float32))

    from concourse.bass_utils import run_bass_kernel_spmd

    nc = _get_compiled()
    in_maps = _make_in_maps(dict(x=x, k_cache=k_cache, v_cache=v_cache, Wq=Wq,
                                 bq=bq, Wk=Wk, Wv=Wv, bv=bv, Wo=Wo, bo=bo))
    res = run_bass_kernel_spmd(nc, in_maps, core_ids=list(range(8))).results

    out = np.empty((B, Tq, D), np.float32)
    kn = np.empty((B, CACHE, D), np.float32)
    vn = np.empty((B, CACHE, D), np.float32)
    for b in range(B):
        acc = res[2 * b]["oo"].astype(np.float32) + res[2 * b + 1]["oo"].astype(np.float32)
        out[b] = acc.T
        for g in (0, 1):
            r = res[2 * b + g]
            kn[b][:, COLS_G[g]] = r["ko"].astype(np.float32).T
            vn[b][:, COLS_G[g]] = r["vo"].astype(np.float32)
    return out, kn, vn


# revision 16
# speedup vs baseline: 1.1407x; 1.1407x over previous
"""ALiBi multi-head attention (B=4, Tq=1024, D=1024, H=16, cache=1024) on 8
Trainium2 NeuronCores.

Sharding: core c = (batch b = c//2, head-group g = c%2). Each core runs one
batch with 8 of the 16 heads. Heads are assigned to groups interleaved by
ALiBi window size so per-core work balances, and all cores run an identical
SPMD graph (per-slot key windows are the max over the two groups).

Device dataflow (all transposed so ALiBi becomes a per-partition ACT bias):
  qT = Wq_s^T x^T + bq        kTnew = Wk_s^T x^T        vnew = x Wv_s + bv
  scoresT[k,q] = kT_chunk^T qT        (K=64 contraction, 2-head row packing)
  P^T = exp(0.125*scoresT + alibi[k])  (single ScalarE pass, PSUM->SBUF bf16)
  oT += v_chunk^T P^T  (V stationary, col-packed pairs -> PSUM[128,1024])
  denom += ones^T P^T  (M=64 ones lhsT -> denominator broadcast to 64 rows)
  wvT = oT * recip(denom);  outT_partial = Wo_s^T wvT + 0.5*bo
Softmax max-subtraction is skipped (scores are O(1), bias <= 0) and keys with
alibi bias < -16 are dropped (error ~1e-7 << tolerance).

Host: shards/pre-transposes inputs (bf16), sums the two partial outT per
batch, scatters head-sliced k/v outputs. No collectives on device.
"""

import numpy as np
import ml_dtypes

BF16 = ml_dtypes.bfloat16

B, Tq, D = 4, 1024, 1024
H, DH = 16, 64
CACHE = 1024
Tk = CACHE + Tq
NCH = Tk // 128  # 16 key chunks
T_CUT = 16.0

# ---- head assignment / windows (hardcoded, deterministic) ----
_slopes = 2.0 ** (-(8.0 / H) * np.arange(1, H + 1))  # head h=0..15 -> slope
_raw = np.minimum(np.ceil(T_CUT / _slopes), Tk)
_W = np.minimum(((_raw + 127) // 128) * 128, Tk).astype(int)
_order = np.argsort(-_W, kind="stable")
SLOT_W = [int(max(_W[_order[2 * s]], _W[_order[2 * s + 1]])) for s in range(8)]
HEADS_OF_GROUP = [[int(_order[2 * s + g]) for s in range(8)] for g in (0, 1)]
COLS_G = [
    np.concatenate([np.arange(h * DH, (h + 1) * DH) for h in HEADS_OF_GROUP[g]])
    for g in (0, 1)
]

_COMPILED = None


def _build():
    import concourse.bacc as bacc
    import concourse.tile as tile
    import concourse.mybir as mybir

    f32 = mybir.dt.float32
    bf16 = mybir.dt.bfloat16
    ADD = mybir.AluOpType.add
    MUL = mybir.AluOpType.mult
    EXP = mybir.ActivationFunctionType.Exp

    nc = bacc.Bacc("TRN2", target_bir_lowering=False, debug=False,
                   num_swdge_queues=4)

    # all inputs pre-arranged on host to [128, free] partition-major layouts
    xt = nc.dram_tensor("xt", [128, 8 * Tq], bf16, kind="ExternalInput")
    wq = nc.dram_tensor("wq", [128, 8 * 512], bf16, kind="ExternalInput")
    wk = nc.dram_tensor("wk", [128, 8 * 512], bf16, kind="ExternalInput")
    wv = nc.dram_tensor("wv", [128, 8 * 512], bf16, kind="ExternalInput")
    wo = nc.dram_tensor("wo", [128, 4 * Tq], bf16, kind="ExternalInput")
    pbf = nc.dram_tensor("pbf", [128, 2048], bf16, kind="ExternalInput")  # vc0|vc1|kct
    pf32 = nc.dram_tensor("pf32", [128, 652], f32, kind="ExternalInput")  # bq|bo|alibi|bvb

    ko = nc.dram_tensor("ko", [512, Tq], bf16, kind="ExternalOutput")
    vo = nc.dram_tensor("vo", [Tq, 512], bf16, kind="ExternalOutput")
    oo = nc.dram_tensor("oo", [D, Tq], bf16, kind="ExternalOutput")

    ko_r = ko[:].rearrange("(c p) t -> p c t", p=128)
    vo_r = vo[:].rearrange("(c p) (s e) -> p c s e", p=128, s=8)
    oo_r = oo[:].rearrange("(c p) t -> p c t", p=128)

    with tile.TileContext(nc) as tc:
        with (
            tc.tile_pool(name="const", bufs=1) as cp,
            tc.tile_pool(name="pt", bufs=4) as ptp,
            tc.tile_pool(name="rc", bufs=2) as rcp,
            tc.tile_pool(name="bc", bufs=2) as bcp,
            tc.tile_pool(name="sc", bufs=2, space="PSUM") as scp,
            tc.tile_pool(name="proj", bufs=1, space="PSUM") as projp,
            tc.tile_pool(name="otp", bufs=1, space="PSUM") as otp,
        ):
            # resident SBUF tensors
            xt_sb = cp.tile([128, 8, Tq], bf16)
            wq_sb = cp.tile([128, 8, 512], bf16)
            wk_sb = cp.tile([128, 8, 512], bf16)
            wv_sb = cp.tile([128, 8, 512], bf16)
            wo_sb = cp.tile([128, 4, Tq], bf16)
            pbf_sb = cp.tile([128, 2048], bf16)
            pf32_sb = cp.tile([128, 652], f32)
            vc_sb = [cp.tile([128, 8, DH + 1], bf16, name=f"vc_sb{i}") for i in (0, 1)]
            qt_sb = cp.tile([128, 4, Tq], bf16)
            kt_sb = cp.tile([128, 4, Tq], bf16)
            v_sb = cp.tile([128, 8, 8, DH + 1], bf16)
            wvt_sb = cp.tile([128, 4, Tq], bf16)
            ot_sb = cp.tile([128, 8, Tq], bf16)

            # zero-copy views into the packed tiles
            kct_sb = pbf_sb[:, 1024:2048]
            bq_sb = pf32_sb[:, 0:4]
            bo_sb = pf32_sb[:, 4:12]
            alibi_sb = pf32_sb[:, 12:140].rearrange("p (s j) -> p s j", s=8)
            bvb_sb = pf32_sb[:, 140:652]

            # loads: stream xt+wq per contraction chunk so the first qT
            # matmul starts ~2us in; everything else queues behind on the
            # three DMA-capable engines.
            xt_r = xt[:].rearrange("p (c t) -> p c t", c=8)
            wq_r = wq[:].rearrange("p (c n) -> p c n", c=8)
            eng = [nc.sync, nc.scalar, nc.gpsimd]
            for kc in range(8):
                eng[kc % 3].dma_start(out=xt_sb[:, kc, :], in_=xt_r[:, kc, :])
                eng[(kc + 1) % 3].dma_start(out=wq_sb[:, kc, :], in_=wq_r[:, kc, :])
            nc.gpsimd.dma_start(out=pf32_sb[:], in_=pf32[:])
            nc.sync.dma_start(out=pbf_sb[:], in_=pbf[:])
            nc.scalar.dma_start(out=wk_sb[:], in_=wk[:].rearrange("p (c n) -> p c n", c=8))
            nc.gpsimd.dma_start(out=wv_sb[:], in_=wv[:].rearrange("p (c n) -> p c n", c=8))
            nc.sync.dma_start(out=wo_sb[:], in_=wo[:].rearrange("p (c n) -> p c n", c=4))
            # unpack cached V into the ones-augmented layout
            for i in (0, 1):
                nc.vector.tensor_copy(
                    out=vc_sb[i][:, :, 0:DH],
                    in_=pbf_sb[:, i * 512:(i + 1) * 512].rearrange("p (s e) -> p s e", s=8))
            # ones columns for the augmented V (softmax denominators)
            nc.vector.memset(v_sb[:, :, :, DH:DH + 1], 1.0)
            nc.vector.memset(vc_sb[0][:, :, DH:DH + 1], 1.0)
            nc.vector.memset(vc_sb[1][:, :, DH:DH + 1], 1.0)

            # ---- qT projection (uses "sc" psum tag; attention not started) ----
            for m in range(4):
                ps = scp.tile([128, 1024], f32, tag="sc")
                for qh in range(2):
                    for kc in range(8):
                        nc.tensor.matmul(
                            ps[:, qh * 512:(qh + 1) * 512],
                            lhsT=wq_sb[:, kc, m * 128:(m + 1) * 128],
                            rhs=xt_sb[:, kc, qh * 512:(qh + 1) * 512],
                            start=(kc == 0), stop=(kc == 7),
                        )
                nc.vector.tensor_scalar(qt_sb[:, m, :], ps[:], bq_sb[:, m:m + 1], None, ADD)

            # ---- filler generators: kT / v projections, pumped during attention ----
            def kt_gen(m):
                ps = projp.tile([128, 1024], f32, tag="proj")
                for qh in range(2):
                    for kc in range(8):
                        nc.tensor.matmul(
                            ps[:, qh * 512:(qh + 1) * 512],
                            lhsT=wk_sb[:, kc, m * 128:(m + 1) * 128],
                            rhs=xt_sb[:, kc, qh * 512:(qh + 1) * 512],
                            start=(kc == 0), stop=(kc == 7),
                        )
                        yield
                nc.vector.tensor_copy(out=kt_sb[:, m, :], in_=ps[:])
                nc.sync.dma_start(out=ko_r[:, m, :], in_=kt_sb[:, m, :])
                yield

            def v_gen(t8):
                ps = projp.tile([128, 1024], f32, tag="proj")
                for kc in range(8):
                    nc.tensor.matmul(
                        ps[:, :512],
                        lhsT=xt_sb[:, kc, t8 * 128:(t8 + 1) * 128],
                        rhs=wv_sb[:, kc, :],
                        start=(kc == 0), stop=(kc == 7),
                    )
                    yield
                nc.vector.tensor_tensor(
                    v_sb[:, t8, :, 0:DH],
                    ps[:, :512].rearrange("p (s e) -> p s e", s=8),
                    bvb_sb[:].rearrange("p (s e) -> p s e", s=8), ADD)
                nc.sync.dma_start(out=vo_r[:, t8, :, :], in_=v_sb[:, t8, :, 0:DH])
                yield

            filler = [kt_gen(0)] + [v_gen(t) for t in range(8)] + \
                     [kt_gen(1), kt_gen(2), kt_gen(3)]

            def pump(n):
                while n > 0 and filler:
                    try:
                        next(filler[0])
                        n -= 1
                    except StopIteration:
                        filler.pop(0)

            # ---- attention: slot-major, AV lags exp by 2 items ----
            for p in range(4):
                for d in (0, 1):
                    s = 2 * p + d
                    W = SLOT_W[s]
                    j_first = NCH - W // 128
                    if s <= 1 or s % 2 == 1:
                        ot = otp.tile([65, 1024], f32, tag="ot")
                    else:
                        ot = projp.tile([65, 1024], f32, tag="proj")

                    def av_emit(j, pt, ot=ot, s=s, p=p, d=d, j_first=j_first):
                        if j < 8:
                            vsrc = vc_sb[s][:, j, :]
                        else:
                            vsrc = v_sb[:, j - 8, s, :]
                        for qh in range(2):
                            nc.tensor.matmul(
                                ot[:, qh * 512:(qh + 1) * 512],
                                lhsT=vsrc,
                                rhs=pt[:, qh * 512:(qh + 1) * 512],
                                start=(j == j_first), stop=(j == NCH - 1),
                                skip_group_check=True,
                            )

                    pend = []
                    for j in range(j_first, NCH):
                        sc = scp.tile([128, 1024], f32, tag="sc")
                        if j < 8:
                            klh = kct_sb[d * 64:(d + 1) * 64, j * 128:(j + 1) * 128]
                        else:
                            klh = kt_sb[d * 64:(d + 1) * 64, p, (j - 8) * 128:(j - 7) * 128]
                        for qh in range(2):
                            nc.tensor.matmul(
                                sc[:, qh * 512:(qh + 1) * 512],
                                lhsT=klh,
                                rhs=qt_sb[d * 64:(d + 1) * 64, p, qh * 512:(qh + 1) * 512],
                                start=True, stop=True,
                            )
                        pt = ptp.tile([128, 1024], bf16, tag="pt")
                        nc.scalar.activation(pt[:], sc[:], EXP,
                                             bias=alibi_sb[:, s, j:j + 1], scale=0.125)
                        pump(6)
                        pend.append((j, pt))
                        if len(pend) > 2:
                            av_emit(*pend.pop(0))
                    for it in pend:
                        av_emit(*it)

                    # normalize: wvT rows = oT * recip(denom row).
                    # DVE rules: tensor_copy may shift base partitions; the
                    # custom recip and 2-input tensor_tensor must be aligned.
                    rc = rcp.tile([1, 1024], f32, tag="rc")
                    rcr = rcp.tile([1, 1024], f32, tag="rcr")
                    nc.vector.tensor_copy(out=rc[:], in_=ot[64:65, :])
                    nc.vector.reciprocal_approx_fast(rcr[:], rc[:])
                    bc = bcp.tile([64, 1024], f32, tag="bc")
                    nc.gpsimd.partition_broadcast(bc[:], rcr[:])
                    if d == 0:
                        nc.vector.tensor_tensor(
                            wvt_sb[0:64, p, :], ot[0:64, :], bc[:], MUL)
                    else:
                        tmp = bcp.tile([64, 1024], bf16, tag="ntmp")
                        nc.vector.tensor_tensor(tmp[:], ot[0:64, :], bc[:], MUL)
                        nc.vector.tensor_copy(out=wvt_sb[64:128, p, :], in_=tmp[:])

            # ---- out projection ----
            for m in range(8):
                ps = scp.tile([128, 1024], f32, tag="sc")
                for qh in range(2):
                    for kc in range(4):
                        nc.tensor.matmul(
                            ps[:, qh * 512:(qh + 1) * 512],
                            lhsT=wo_sb[:, kc, m * 128:(m + 1) * 128],
                            rhs=wvt_sb[:, kc, qh * 512:(qh + 1) * 512],
                            start=(kc == 0), stop=(kc == 3),
                        )
                nc.vector.tensor_scalar(ot_sb[:, m, :], ps[:], bo_sb[:, m:m + 1], None, ADD)
                nc.sync.dma_start(out=oo_r[:, m, :], in_=ot_sb[:, m, :])

    nc.compile()
    return nc


def _get_compiled():
    global _COMPILED
    if _COMPILED is None:
        _COMPILED = _build()
    return _COMPILED


def _reference_numpy(x, k_cache, v_cache, mask, Wq, bq, Wk, Wv, bv, Wo, bo):
    """Exact numpy fallback (used only if mask is nonzero)."""
    q = x @ Wq + bq
    k = np.concatenate([k_cache, x @ Wk], axis=1)
    v = np.concatenate([v_cache, x @ Wv + bv], axis=1)
    kn, vn = k[:, -CACHE:, :], v[:, -CACHE:, :]
    qh = q.reshape(B, Tq, H, DH).transpose(0, 2, 1, 3)
    kh = k.reshape(B, Tk, H, DH).transpose(0, 2, 1, 3)
    vh = v.reshape(B, Tk, H, DH).transpose(0, 2, 1, 3)
    slopes = 2.0 ** (-(8.0 / H) * np.arange(1, H + 1))
    rel = np.arange(Tk - 1, -1, -1, dtype=np.float32)
    bias = (-(slopes[:, None] * rel[None, :])).astype(np.float32)[None, :, None, :]
    scores = np.einsum("bhqd,bhkd->bhqk", qh, kh) / np.sqrt(DH) + mask + bias
    scores -= scores.max(axis=-1, keepdims=True)
    e = np.exp(scores)
    attn = e / e.sum(axis=-1, keepdims=True)
    a = np.einsum("bhqk,bhkd->bhqd", attn, vh)
    out = a.transpose(0, 2, 1, 3).reshape(B, Tq, D) @ Wo + bo
    return (out.astype(np.float32), kn.astype(np.float32), vn.astype(np.float32))


def _make_in_maps(inputs):
    x = np.asarray(inputs["x"], np.float32)
    k_cache = np.asarray(inputs["k_cache"], np.float32)
    v_cache = np.asarray(inputs["v_cache"], np.float32)
    Wq, bq = np.asarray(inputs["Wq"], np.float32), np.asarray(inputs["bq"], np.float32)
    Wk = np.asarray(inputs["Wk"], np.float32)
    Wv, bv = np.asarray(inputs["Wv"], np.float32), np.asarray(inputs["bv"], np.float32)
    Wo, bo = np.asarray(inputs["Wo"], np.float32), np.asarray(inputs["bo"], np.float32)

    def pmajor(a, nch):
        # (nch*128, F) -> (128, nch*F) partition-major
        F = a.shape[1]
        return np.ascontiguousarray(
            a.reshape(nch, 128, F).transpose(1, 0, 2).reshape(128, nch * F))

    alibi_g, pf32_g = [], []
    for g in (0, 1):
        heads = HEADS_OF_GROUP[g]
        al = np.empty((128, 8, NCH), np.float32)
        kpos = np.arange(128)
        for s in range(8):
            sl = _slopes[heads[s]]
            for j in range(NCH):
                al[:, s, j] = -sl * (Tk - 1 - (j * 128 + kpos))
        alibi_g.append(al)
        cols = COLS_G[g]
        pf = np.empty((128, 652), np.float32)
        pf[:, 0:4] = bq[cols].reshape(4, 128).T
        pf[:, 4:12] = (0.5 * bo).reshape(8, 128).T
        pf[:, 12:140] = al.reshape(128, 128)
        pf[:, 140:652] = np.broadcast_to(bv[cols], (128, 512))
        pf32_g.append(pf)

    in_maps = []
    for c in range(8):
        b, g = c // 2, c % 2
        heads = HEADS_OF_GROUP[g]
        cols = COLS_G[g]
        kct_arr = np.concatenate(
            [k_cache[b][:, heads[s] * DH:(heads[s] + 1) * DH].T for s in (0, 1)], axis=0
        ).astype(BF16)  # (128, 1024)
        pb = np.empty((128, 2048), BF16)
        for i in (0, 1):
            vcs = v_cache[b][:, heads[i] * DH:(heads[i] + 1) * DH].astype(BF16)  # (1024, 64)
            pb[:, i * 512:(i + 1) * 512] = (
                vcs.reshape(8, 128, DH).transpose(1, 0, 2).reshape(128, 512))
        pb[:, 1024:2048] = kct_arr
        in_maps.append({
            "xt": pmajor(np.ascontiguousarray(x[b].T).astype(BF16), 8),
            "wq": pmajor(Wq[:, cols].astype(BF16), 8),
            "wk": pmajor(Wk[:, cols].astype(BF16), 8),
            "wv": pmajor(Wv[:, cols].astype(BF16), 8),
            "wo": pmajor(Wo[cols, :].astype(BF16), 4),
            "pbf": pb,
            "pf32": pf32_g[g],
        })
    return in_maps


def kernel(x, k_cache, v_cache, mask, Wq, bq, Wk, Wv, bv, Wo, bo):
    mask = np.asarray(mask, np.float32)
    if np.any(mask):
        return _reference_numpy(
            np.asarray(x, np.float32), np.asarray(k_cache, np.float32),
            np.asarray(v_cache, np.float32), mask,
            np.asarray(Wq, np.float32), np.asarray(bq, np.float32),
            np.asarray(Wk, np.float32), np.asarray(Wv, np.float32),
            np.asarray(bv, np.float32), np.asarray(Wo, np.float32),
            np.asarray(bo, np.float32))

    from concourse.bass_utils import run_bass_kernel_spmd

    nc = _get_compiled()
    in_maps = _make_in_maps(dict(x=x, k_cache=k_cache, v_cache=v_cache, Wq=Wq,
                                 bq=bq, Wk=Wk, Wv=Wv, bv=bv, Wo=Wo, bo=bo))
    res = run_bass_kernel_spmd(nc, in_maps, core_ids=list(range(8))).results

    out = np.empty((B, Tq, D), np.float32)
    kn = np.empty((B, CACHE, D), np.float32)
    vn = np.empty((B, CACHE, D), np.float32)
    for b in range(B):
        acc = res[2 * b]["oo"].astype(np.float32) + res[2 * b + 1]["oo"].astype(np.float32)
        out[b] = acc.T
        for g in (0, 1):
            r = res[2 * b + g]
            kn[b][:, COLS_G[g]] = r["ko"].astype(np.float32).T
            vn[b][:, COLS_G[g]] = r["vo"].astype(np.float32)
    return out, kn, vn


# revision 18
# speedup vs baseline: 1.2308x; 1.0789x over previous
"""ALiBi multi-head attention (B=4, Tq=1024, D=1024, H=16, cache=1024) on 8
Trainium2 NeuronCores.

Sharding: core c = (batch b = c//2, head-group g = c%2). Each core runs one
batch with 8 of the 16 heads. Heads are assigned to groups interleaved by
ALiBi window size so per-core work balances, and all cores run an identical
SPMD graph (per-slot key windows are the max over the two groups).

Device dataflow (all transposed so ALiBi becomes a per-partition ACT bias):
  qT = Wq_s^T x^T + bq        kTnew = Wk_s^T x^T        vnew = x Wv_s + bv
  scoresT[k,q] = kT_chunk^T qT        (K=64 contraction, 2-head row packing)
  P^T = exp(0.125*scoresT + alibi[k])  (single ScalarE pass, PSUM->SBUF bf16)
  oT += v_chunk^T P^T  (V stationary, col-packed pairs -> PSUM[128,1024])
  denom += ones^T P^T  (M=64 ones lhsT -> denominator broadcast to 64 rows)
  wvT = oT * recip(denom);  outT_partial = Wo_s^T wvT + 0.5*bo
Softmax max-subtraction is skipped (scores are O(1), bias <= 0) and keys with
alibi bias < -16 are dropped (error ~1e-7 << tolerance).

Host: shards/pre-transposes inputs (bf16), sums the two partial outT per
batch, scatters head-sliced k/v outputs. No collectives on device.
"""

import numpy as np
import ml_dtypes

BF16 = ml_dtypes.bfloat16

B, Tq, D = 4, 1024, 1024
H, DH = 16, 64
CACHE = 1024
Tk = CACHE + Tq
NCH = Tk // 128  # 16 key chunks
T_CUT = 16.0

# ---- head assignment / windows (hardcoded, deterministic) ----
_slopes = 2.0 ** (-(8.0 / H) * np.arange(1, H + 1))  # head h=0..15 -> slope
_raw = np.minimum(np.ceil(T_CUT / _slopes), Tk)
_W = np.minimum(((_raw + 127) // 128) * 128, Tk).astype(int)
_order = np.argsort(-_W, kind="stable")
SLOT_W = [int(max(_W[_order[2 * s]], _W[_order[2 * s + 1]])) for s in range(8)]
HEADS_OF_GROUP = [[int(_order[2 * s + g]) for s in range(8)] for g in (0, 1)]
COLS_G = [
    np.concatenate([np.arange(h * DH, (h + 1) * DH) for h in HEADS_OF_GROUP[g]])
    for g in (0, 1)
]

_COMPILED = None


def _build():
    import concourse.bacc as bacc
    import concourse.tile as tile
    import concourse.mybir as mybir

    f32 = mybir.dt.float32
    bf16 = mybir.dt.bfloat16
    ADD = mybir.AluOpType.add
    MUL = mybir.AluOpType.mult
    EXP = mybir.ActivationFunctionType.Exp

    nc = bacc.Bacc("TRN2", target_bir_lowering=False, debug=False,
                   num_swdge_queues=4)

    # all inputs pre-arranged on host to [128, free] partition-major layouts
    xt = nc.dram_tensor("xt", [128, 8 * Tq], bf16, kind="ExternalInput")
    wq = nc.dram_tensor("wq", [128, 8 * 512], bf16, kind="ExternalInput")
    wk = nc.dram_tensor("wk", [128, 8 * 512], bf16, kind="ExternalInput")
    wv = nc.dram_tensor("wv", [128, 8 * 512], bf16, kind="ExternalInput")
    wo = nc.dram_tensor("wo", [128, 4 * Tq], bf16, kind="ExternalInput")
    pbf = nc.dram_tensor("pbf", [128, 2048], bf16, kind="ExternalInput")  # vc0|vc1|kct
    pf32 = nc.dram_tensor("pf32", [128, 652], f32, kind="ExternalInput")  # bq|bo|alibi|bvb

    ko = nc.dram_tensor("ko", [512, Tq], bf16, kind="ExternalOutput")
    vo = nc.dram_tensor("vo", [Tq, 512], bf16, kind="ExternalOutput")
    oo = nc.dram_tensor("oo", [D, Tq], bf16, kind="ExternalOutput")

    ko_r = ko[:].rearrange("(c p) t -> p c t", p=128)
    vo_r = vo[:].rearrange("(c p) (s e) -> p c s e", p=128, s=8)
    oo_r = oo[:].rearrange("(c p) t -> p c t", p=128)

    with tile.TileContext(nc) as tc:
        with (
            tc.tile_pool(name="const", bufs=1) as cp,
            tc.tile_pool(name="pt", bufs=4) as ptp,
            tc.tile_pool(name="rc", bufs=2) as rcp,
            tc.tile_pool(name="bc", bufs=2) as bcp,
            tc.tile_pool(name="sc", bufs=2, space="PSUM") as scp,
            tc.tile_pool(name="proj", bufs=1, space="PSUM") as projp,
            tc.tile_pool(name="otp", bufs=1, space="PSUM") as otp,
        ):
            # resident SBUF tensors
            xt_sb = cp.tile([128, 8, Tq], bf16)
            wq_sb = cp.tile([128, 8, 512], bf16)
            wk_sb = cp.tile([128, 8, 512], bf16)
            wv_sb = cp.tile([128, 8, 512], bf16)
            wo_sb = cp.tile([128, 4, Tq], bf16)
            pbf_sb = cp.tile([128, 2048], bf16)
            pf32_sb = cp.tile([128, 652], f32)
            vc_sb = [cp.tile([128, 8, DH + 1], bf16, name=f"vc_sb{i}") for i in (0, 1)]
            qt_sb = cp.tile([128, 4, Tq], bf16)
            kt_sb = cp.tile([128, 4, Tq], bf16)
            v_sb = cp.tile([128, 8, 8, DH + 1], bf16)
            wvt_sb = cp.tile([128, 4, Tq], bf16)
            ot_sb = cp.tile([128, 8, Tq], bf16)
            ones64 = cp.tile([128, 64], bf16)

            # zero-copy views into the packed tiles
            kct_sb = pbf_sb[:, 1024:2048]
            bq_sb = pf32_sb[:, 0:4]
            bo_sb = pf32_sb[:, 4:12]
            alibi_sb = pf32_sb[:, 12:140].rearrange("p (s j) -> p s j", s=8)
            bvb_sb = pf32_sb[:, 140:652]

            # loads: stream xt+wq per contraction chunk so the first qT
            # matmul starts ~2us in; everything else queues behind on the
            # three DMA-capable engines.
            xt_r = xt[:].rearrange("p (c t) -> p c t", c=8)
            wq_r = wq[:].rearrange("p (c n) -> p c n", c=8)
            eng = [nc.sync, nc.scalar, nc.gpsimd]
            for kc in range(8):
                eng[kc % 3].dma_start(out=xt_sb[:, kc, :], in_=xt_r[:, kc, :])
                eng[(kc + 1) % 3].dma_start(out=wq_sb[:, kc, :], in_=wq_r[:, kc, :])
            nc.gpsimd.dma_start(out=pf32_sb[:], in_=pf32[:])
            nc.sync.dma_start(out=pbf_sb[:], in_=pbf[:])
            nc.scalar.dma_start(out=wk_sb[:], in_=wk[:].rearrange("p (c n) -> p c n", c=8))
            nc.gpsimd.dma_start(out=wv_sb[:], in_=wv[:].rearrange("p (c n) -> p c n", c=8))
            nc.sync.dma_start(out=wo_sb[:], in_=wo[:].rearrange("p (c n) -> p c n", c=4))
            # unpack cached V into the ones-augmented layout
            for i in (0, 1):
                nc.vector.tensor_copy(
                    out=vc_sb[i][:, :, 0:DH],
                    in_=pbf_sb[:, i * 512:(i + 1) * 512].rearrange("p (s e) -> p s e", s=8))
            # ones columns for the augmented V (softmax denominators)
            nc.vector.memset(ones64[:], 1.0)
            nc.vector.memset(v_sb[:, :, :, DH:DH + 1], 1.0)
            nc.vector.memset(vc_sb[0][:, :, DH:DH + 1], 1.0)
            nc.vector.memset(vc_sb[1][:, :, DH:DH + 1], 1.0)

            # ---- qT projection (uses "sc" psum tag; attention not started) ----
            for m in range(4):
                ps = scp.tile([128, 1024], f32, tag="sc")
                for qh in range(2):
                    for kc in range(8):
                        nc.tensor.matmul(
                            ps[:, qh * 512:(qh + 1) * 512],
                            lhsT=wq_sb[:, kc, m * 128:(m + 1) * 128],
                            rhs=xt_sb[:, kc, qh * 512:(qh + 1) * 512],
                            start=(kc == 0), stop=(kc == 7),
                        )
                nc.vector.tensor_scalar(qt_sb[:, m, :], ps[:], bq_sb[:, m:m + 1], None, ADD)

            # ---- filler generators: kT / v projections, pumped during attention ----
            def kt_gen(m):
                ps = projp.tile([128, 1024], f32, tag="proj")
                for qh in range(2):
                    for kc in range(8):
                        nc.tensor.matmul(
                            ps[:, qh * 512:(qh + 1) * 512],
                            lhsT=wk_sb[:, kc, m * 128:(m + 1) * 128],
                            rhs=xt_sb[:, kc, qh * 512:(qh + 1) * 512],
                            start=(kc == 0), stop=(kc == 7),
                        )
                        yield
                nc.vector.tensor_copy(out=kt_sb[:, m, :], in_=ps[:])
                nc.sync.dma_start(out=ko_r[:, m, :], in_=kt_sb[:, m, :])
                yield

            def v_gen(t8):
                ps = projp.tile([128, 1024], f32, tag="proj")
                for kc in range(8):
                    nc.tensor.matmul(
                        ps[:, :512],
                        lhsT=xt_sb[:, kc, t8 * 128:(t8 + 1) * 128],
                        rhs=wv_sb[:, kc, :],
                        start=(kc == 0), stop=(kc == 7),
                    )
                    yield
                nc.vector.tensor_tensor(
                    v_sb[:, t8, :, 0:DH],
                    ps[:, :512].rearrange("p (s e) -> p s e", s=8),
                    bvb_sb[:].rearrange("p (s e) -> p s e", s=8), ADD)
                nc.sync.dma_start(out=vo_r[:, t8, :, :], in_=v_sb[:, t8, :, 0:DH])
                yield

            filler = [kt_gen(0)] + [v_gen(t) for t in range(8)] + \
                     [kt_gen(1), kt_gen(2), kt_gen(3)]

            def pump(n):
                while n > 0 and filler:
                    try:
                        next(filler[0])
                        n -= 1
                    except StopIteration:
                        filler.pop(0)

            # ---- attention ----
            # pair 0 (big windows): slot-major with ones-augmented V; the
            # denominator rides along as PSUM row 64 and the normalize chain
            # amortizes over 32 items (hidden under proj filler).
            for d in (0, 1):
                s = d
                W = SLOT_W[s]
                j_first = NCH - W // 128
                ot = otp.tile([65, 1024], f32, tag="ot")

                def av_emit(j, pt, ot=ot, s=s, j_first=j_first):
                    if j < 8:
                        vsrc = vc_sb[s][:, j, :]
                    else:
                        vsrc = v_sb[:, j - 8, s, :]
                    for qh in range(2):
                        nc.tensor.matmul(
                            ot[:, qh * 512:(qh + 1) * 512],
                            lhsT=vsrc,
                            rhs=pt[:, qh * 512:(qh + 1) * 512],
                            start=(j == j_first), stop=(j == NCH - 1),
                            skip_group_check=True,
                        )

                pend = []
                for j in range(j_first, NCH):
                    sc = scp.tile([128, 1024], f32, tag="sc")
                    if j < 8:
                        klh = kct_sb[d * 64:(d + 1) * 64, j * 128:(j + 1) * 128]
                    else:
                        klh = kt_sb[d * 64:(d + 1) * 64, 0, (j - 8) * 128:(j - 7) * 128]
                    for qh in range(2):
                        nc.tensor.matmul(
                            sc[:, qh * 512:(qh + 1) * 512],
                            lhsT=klh,
                            rhs=qt_sb[d * 64:(d + 1) * 64, 0, qh * 512:(qh + 1) * 512],
                            start=True, stop=True,
                        )
                    pt = ptp.tile([128, 1024], bf16, tag="pt")
                    nc.scalar.activation(pt[:], sc[:], EXP,
                                         bias=alibi_sb[:, s, j:j + 1], scale=0.125)
                    pump(6)
                    pend.append((j, pt))
                    if len(pend) > 2:
                        av_emit(*pend.pop(0))
                for it in pend:
                    av_emit(*it)

                # normalize (DVE shift rules: copy may shift partitions,
                # recip/tensor_tensor must stay base-aligned)
                rc = rcp.tile([1, 1024], f32, tag="rc")
                rcr = rcp.tile([1, 1024], f32, tag="rcr")
                nc.vector.tensor_copy(out=rc[:], in_=ot[64:65, :])
                nc.vector.reciprocal_approx_fast(rcr[:], rc[:])
                bc = bcp.tile([64, 1024], f32, tag="bc")
                nc.gpsimd.partition_broadcast(bc[:], rcr[:])
                if d == 0:
                    nc.vector.tensor_tensor(
                        wvt_sb[0:64, 0, :], ot[0:64, :], bc[:], MUL)
                else:
                    tmp = bcp.tile([64, 1024], bf16, tag="ntmp")
                    nc.vector.tensor_tensor(tmp[:], ot[0:64, :], bc[:], MUL)
                    nc.vector.tensor_copy(out=wvt_sb[64:128, 0, :], in_=tmp[:])

            # pairs 1-3 (small windows): chunk-major, col-packed AV pairs plus
            # ones-matmul denominators broadcast in PSUM rows -> 2-op normalize
            for p in range(1, 4):
                Wmax = SLOT_W[2 * p]
                jp = NCH - Wmax // 128
                if p % 2 == 0:
                    ot = otp.tile([128, 1024], f32, tag="ot")
                    dn = projp.tile([128, 1024], f32, tag="proj")
                else:
                    ot = projp.tile([128, 1024], f32, tag="proj")
                    dn = otp.tile([128, 1024], f32, tag="ot")

                def av2(j, d, pt, ot=ot, dn=dn, p=p):
                    s = 2 * p + d
                    first = j == NCH - SLOT_W[s] // 128
                    vsrc = v_sb[:, j - 8, s, 0:DH]
                    for qh in range(2):
                        nc.tensor.matmul(
                            ot[d * 64:(d + 1) * 64, qh * 512:(qh + 1) * 512],
                            lhsT=vsrc,
                            rhs=pt[:, qh * 512:(qh + 1) * 512],
                            start=first, stop=(j == NCH - 1),
                            tile_position=(0, d * 64),
                            skip_group_check=True,
                        )
                        nc.tensor.matmul(
                            dn[d * 64:(d + 1) * 64, qh * 512:(qh + 1) * 512],
                            lhsT=ones64[:],
                            rhs=pt[:, qh * 512:(qh + 1) * 512],
                            start=first, stop=(j == NCH - 1),
                            tile_position=(0, d * 64),
                            skip_group_check=True,
                        )

                pend = []
                for j in range(jp, NCH):
                    for d in (0, 1):
                        s = 2 * p + d
                        if j < NCH - SLOT_W[s] // 128:
                            continue
                        sc = scp.tile([128, 1024], f32, tag="sc")
                        klh = kt_sb[d * 64:(d + 1) * 64, p, (j - 8) * 128:(j - 7) * 128]
                        for qh in range(2):
                            nc.tensor.matmul(
                                sc[:, qh * 512:(qh + 1) * 512],
                                lhsT=klh,
                                rhs=qt_sb[d * 64:(d + 1) * 64, p, qh * 512:(qh + 1) * 512],
                                start=True, stop=True,
                            )
                        pt = ptp.tile([128, 1024], bf16, tag="pt")
                        nc.scalar.activation(pt[:], sc[:], EXP,
                                             bias=alibi_sb[:, s, j:j + 1], scale=0.125)
                        pump(2)
                        pend.append((j, d, pt))
                        if len(pend) > 2:
                            av2(*pend.pop(0))
                for it in pend:
                    av2(*it)

                rc128 = bcp.tile([128, 1024], f32, tag="rc128")
                nc.vector.reciprocal_approx_fast(rc128[:], dn[:])
                nc.vector.tensor_tensor(wvt_sb[:, p, :], ot[:], rc128[:], MUL)

            # ---- out projection ----
            for m in range(8):
                ps = scp.tile([128, 1024], f32, tag="sc")
                for qh in range(2):
                    for kc in range(4):
                        nc.tensor.matmul(
                            ps[:, qh * 512:(qh + 1) * 512],
                            lhsT=wo_sb[:, kc, m * 128:(m + 1) * 128],
                            rhs=wvt_sb[:, kc, qh * 512:(qh + 1) * 512],
                            start=(kc == 0), stop=(kc == 3),
                        )
                nc.vector.tensor_scalar(ot_sb[:, m, :], ps[:], bo_sb[:, m:m + 1], None, ADD)
                nc.sync.dma_start(out=oo_r[:, m, :], in_=ot_sb[:, m, :])

    nc.compile()
    return nc


def _get_compiled():
    global _COMPILED
    if _COMPILED is None:
        _COMPILED = _build()
    return _COMPILED


def _reference_numpy(x, k_cache, v_cache, mask, Wq, bq, Wk, Wv, bv, Wo, bo):
    """Exact numpy fallback (used only if mask is nonzero)."""
    q = x @ Wq + bq
    k = np.concatenate([k_cache, x @ Wk], axis=1)
    v = np.concatenate([v_cache, x @ Wv + bv], axis=1)
    kn, vn = k[:, -CACHE:, :], v[:, -CACHE:, :]
    qh = q.reshape(B, Tq, H, DH).transpose(0, 2, 1, 3)
    kh = k.reshape(B, Tk, H, DH).transpose(0, 2, 1, 3)
    vh = v.reshape(B, Tk, H, DH).transpose(0, 2, 1, 3)
    slopes = 2.0 ** (-(8.0 / H) * np.arange(1, H + 1))
    rel = np.arange(Tk - 1, -1, -1, dtype=np.float32)
    bias = (-(slopes[:, None] * rel[None, :])).astype(np.float32)[None, :, None, :]
    scores = np.einsum("bhqd,bhkd->bhqk", qh, kh) / np.sqrt(DH) + mask + bias
    scores -= scores.max(axis=-1, keepdims=True)
    e = np.exp(scores)
    attn = e / e.sum(axis=-1, keepdims=True)
    a = np.einsum("bhqk,bhkd->bhqd", attn, vh)
    out = a.transpose(0, 2, 1, 3).reshape(B, Tq, D) @ Wo + bo
    return (out.astype(np.float32), kn.astype(np.float32), vn.astype(np.float32))


def _make_in_maps(inputs):
    x = np.asarray(inputs["x"], np.float32)
    k_cache = np.asarray(inputs["k_cache"], np.float32)
    v_cache = np.asarray(inputs["v_cache"], np.float32)
    Wq, bq = np.asarray(inputs["Wq"], np.float32), np.asarray(inputs["bq"], np.float32)
    Wk = np.asarray(inputs["Wk"], np.float32)
    Wv, bv = np.asarray(inputs["Wv"], np.float32), np.asarray(inputs["bv"], np.float32)
    Wo, bo = np.asarray(inputs["Wo"], np.float32), np.asarray(inputs["bo"], np.float32)

    def pmajor(a, nch):
        # (nch*128, F) -> (128, nch*F) partition-major
        F = a.shape[1]
        return np.ascontiguousarray(
            a.reshape(nch, 128, F).transpose(1, 0, 2).reshape(128, nch * F))

    alibi_g, pf32_g = [], []
    for g in (0, 1):
        heads = HEADS_OF_GROUP[g]
        al = np.empty((128, 8, NCH), np.float32)
        kpos = np.arange(128)
        for s in range(8):
            sl = _slopes[heads[s]]
            for j in range(NCH):
                al[:, s, j] = -sl * (Tk - 1 - (j * 128 + kpos))
        alibi_g.append(al)
        cols = COLS_G[g]
        pf = np.empty((128, 652), np.float32)
        pf[:, 0:4] = bq[cols].reshape(4, 128).T
        pf[:, 4:12] = (0.5 * bo).reshape(8, 128).T
        pf[:, 12:140] = al.reshape(128, 128)
        pf[:, 140:652] = np.broadcast_to(bv[cols], (128, 512))
        pf32_g.append(pf)

    in_maps = []
    for c in range(8):
        b, g = c // 2, c % 2
        heads = HEADS_OF_GROUP[g]
        cols = COLS_G[g]
        kct_arr = np.concatenate(
            [k_cache[b][:, heads[s] * DH:(heads[s] + 1) * DH].T for s in (0, 1)], axis=0
        ).astype(BF16)  # (128, 1024)
        pb = np.empty((128, 2048), BF16)
        for i in (0, 1):
            vcs = v_cache[b][:, heads[i] * DH:(heads[i] + 1) * DH].astype(BF16)  # (1024, 64)
            pb[:, i * 512:(i + 1) * 512] = (
                vcs.reshape(8, 128, DH).transpose(1, 0, 2).reshape(128, 512))
        pb[:, 1024:2048] = kct_arr
        in_maps.append({
            "xt": pmajor(np.ascontiguousarray(x[b].T).astype(BF16), 8),
            "wq": pmajor(Wq[:, cols].astype(BF16), 8),
            "wk": pmajor(Wk[:, cols].astype(BF16), 8),
            "wv": pmajor(Wv[:, cols].astype(BF16), 8),
            "wo": pmajor(Wo[cols, :].astype(BF16), 4),
            "pbf": pb,
            "pf32": pf32_g[g],
        })
    return in_maps


def kernel(x, k_cache, v_cache, mask, Wq, bq, Wk, Wv, bv, Wo, bo):
    mask = np.asarray(mask, np.float32)
    if np.any(mask):
        return _reference_numpy(
            np.asarray(x, np.float32), np.asarray(k_cache, np.float32),
            np.asarray(v_cache, np.float32), mask,
            np.asarray(Wq, np.float32), np.asarray(bq, np.float32),
            np.asarray(Wk, np.float32), np.asarray(Wv, np.float32),
            np.asarray(bv, np.float32), np.asarray(Wo, np.float32),
            np.asarray(bo, np.float32))

    from concourse.bass_utils import run_bass_kernel_spmd

    nc = _get_compiled()
    in_maps = _make_in_maps(dict(x=x, k_cache=k_cache, v_cache=v_cache, Wq=Wq,
                                 bq=bq, Wk=Wk, Wv=Wv, bv=bv, Wo=Wo, bo=bo))
    res = run_bass_kernel_spmd(nc, in_maps, core_ids=list(range(8))).results

    out = np.empty((B, Tq, D), np.float32)
    kn = np.empty((B, CACHE, D), np.float32)
    vn = np.empty((B, CACHE, D), np.float32)
    for b in range(B):
        acc = res[2 * b]["oo"].astype(np.float32) + res[2 * b + 1]["oo"].astype(np.float32)
        out[b] = acc.T
        for g in (0, 1):
            r = res[2 * b + g]
            kn[b][:, COLS_G[g]] = r["ko"].astype(np.float32).T
            vn[b][:, COLS_G[g]] = r["vo"].astype(np.float32)
    return out, kn, vn
